# revision 2
# baseline (speedup 1.0000x reference)
"""Trainium2 Bass kernel for AllAtomEnergyBranch (3-layer MLP over broadcast concat).

Math (per batch b, position n, edge e):
    out[b,n,e,0] = W3^T relu(W2^T relu(Wh^T h[b,n] + We^T e_feat[e] + b1) + b2) + b3

Sharding: data-parallel over B (8 batches -> 8 NeuronCores), weights replicated.
Each core computes its [64, 256] output slice independently; no collectives.

Per-core dataflow ("s3z" scheme, software-pipelined):
  - preamble: epT [512k, 256e] = We_aug.T @ eT_aug (b1 folded via ones row),
    hpT [512k, 64n] = Wh.T @ hT.  X(0) built before the block loop.
  - hidden-2 units are PERMUTED on host so that within each PE lane p the
    three units assigned to j-tiles 0..2 share one sign s(p); |w3| (and
    |w3|*b2) are folded into the ACT relu scale/bias for those tiles, and
    the sign rides the [128,1] +-1 stationary of the per-block reduce
    matmul.  Tile 3 takes the leftover mixed-sign units: plain relu on ACT,
    then one DVE multiply by (w3*s) so the sign cancels in the reduce.
    This removes 3 of the 4 per-block DVE w3-multiplies of the baseline at
    zero PE cost (still exactly one reduce matmul per block) and works for
    any sign pattern / any b2.
  - per block of 2 n's (32 blocks), emission software-pipelined:
      j-order (3,0,1,2); X(b+1) DVE builds hoisted between; reduce-MM(b-1)
      interleaved after block b's second matmul group; output drained via
      ACT once per 4 blocks from a strip-batched PSUM tile (ones_ct=4
      pattern: reduce matmuls target partitions 0/32/64/96 by
      tile_position).
  - engine budget per block (spec): PE 17 matmuls ~3.7us, ACT 4 relus +
    out/4 ~2.4us, DVE 8 X-builds + 1 mult + 3 adds ~2.2us.  PE-bound.

Measured (this session's bench methodology: interleaved min-wall delta of
For_i R=32 vs R=1024 NEFFs): baseline (previous session's kernel) 148.8us,
this kernel 113.8us (TimelineSim single-shot 100.2us vs baseline 131.1us).
rel err vs f32 reference: 0.0049 (harness gate 2e-2).
"""

import numpy as np
import ml_dtypes

import concourse.bass as bass
import concourse.mybir as mybir
from concourse import bacc
from concourse.bass import ts
from concourse.tile import TileContext
from concourse.bass_utils import run_bass_kernel_spmd

BF16 = mybir.dt.bfloat16
F32 = mybir.dt.float32

B, N, H = 8, 64, 256
NE, E = 256, 64
HID, OUT = 512, 1
KT = HID // 128   # 4 k-tiles of layer-1 output / layer-2 contraction
JT = HID // 128   # 4 j-tiles of layer-2 output / layer-3 contraction
HT = H // 128     # 2 h-tiles of layer-1 contraction
NBLK = N // 2     # blocks of 2 n-values -> 512 moving columns per matmul
OG = 4            # output blocks batched per PSUM strip-tile / ACT drain


def build(nc, repeat=1, dyn_repeat=None, y_bufs=6, x_bufs=2, yt_bufs=2,
          red_at=1, warm=8):
    """Build the per-core graph (s3z scheme). All 8 cores run the same
    program.

    repeat / dyn_repeat: repeat the whole computation inside the NEFF
    (python-unrolled / For_i hardware loop) -- benchmarking only.
    red_at: which j-group of block b+1 the reduce-MM of block b is emitted
    after (0..3); ssum(b) lands ~1 group into block b+1.
    """
    ht_d = nc.declare_dram_parameter("ht", [HT, 128, N], BF16, isOutput=False)
    wh_d = nc.declare_dram_parameter("wh", [HT, 128, HID], BF16, isOutput=False)
    we_d = nc.declare_dram_parameter("we", [E + 1, HID], BF16, isOutput=False)
    et_d = nc.declare_dram_parameter("et", [E + 1, NE], BF16, isOutput=False)
    w2_d = nc.declare_dram_parameter("w2", [KT, 128, HID], BF16, isOutput=False)
    scl_d = nc.declare_dram_parameter("scl", [128, JT], F32, isOutput=False)
    bias2_d = nc.declare_dram_parameter("bias2", [128, JT], F32, isOutput=False)
    w3s3_d = nc.declare_dram_parameter("w3s3", [128, 1], F32, isOutput=False)
    vst_d = nc.declare_dram_parameter("vst", [128, 1], BF16, isOutput=False)
    b3_d = nc.declare_dram_parameter("b3", [128, 1], F32, isOutput=False)
    out_d = nc.declare_dram_parameter("out", [NBLK, 512], F32, isOutput=True)

    relu = mybir.ActivationFunctionType.Relu
    ident = mybir.ActivationFunctionType.Identity
    add = mybir.AluOpType.add
    mult = mybir.AluOpType.mult
    amax = mybir.AluOpType.max

    JORD = (3, 0, 1, 2)   # tile 3 first so z3 and the add chain start early

    with TileContext(nc) as tc:
        with (
            tc.tile_pool(name="const", bufs=1) as cpool,
            tc.tile_pool(name="xp", bufs=x_bufs) as xpool,
            tc.tile_pool(name="yp", bufs=yt_bufs) as ypool,
            tc.tile_pool(name="sp", bufs=3) as spool,
            tc.tile_pool(name="op", bufs=2) as opool,
            tc.tile_pool(name="psY", bufs=y_bufs, space="PSUM") as y_ps,
            tc.tile_pool(name="psO", bufs=2, space="PSUM") as o_ps,
        ):
            # ---- load weights / inputs into SBUF ----
            # Preamble operands (we/et/ht/wh) first so the PE can start
            # while W2 is still in flight.
            we_t = cpool.tile([E + 1, HID], BF16, tag="we")
            nc.sync.dma_start(out=we_t[:], in_=we_d[:])
            et_t = cpool.tile([E + 1, NE], BF16, tag="et")
            nc.sync.dma_start(out=et_t[:], in_=et_d[:])
            ht_t = []
            for h in range(HT):
                t = cpool.tile([128, N], BF16, tag=f"ht{h}", name=f"ht{h}")
                nc.sync.dma_start(out=t[:], in_=ht_d[h])
                ht_t.append(t)
            wh_t = []
            for h in range(HT):
                t = cpool.tile([128, HID], BF16, tag=f"wh{h}", name=f"wh{h}")
                nc.sync.dma_start(out=t[:], in_=wh_d[h])
                wh_t.append(t)
            scl_t = cpool.tile([128, JT], F32, tag="scl")
            nc.sync.dma_start(out=scl_t[:], in_=scl_d[:])
            bias2_t = cpool.tile([128, JT], F32, tag="bias2")
            nc.sync.dma_start(out=bias2_t[:], in_=bias2_d[:])
            w3s3_t = cpool.tile([128, 1], F32, tag="w3s3")
            nc.sync.dma_start(out=w3s3_t[:], in_=w3s3_d[:])
            vst_t = cpool.tile([128, 1], BF16, tag="vst")
            nc.sync.dma_start(out=vst_t[:], in_=vst_d[:])
            b3_t = cpool.tile([128, 1], F32, tag="b3")
            nc.sync.dma_start(out=b3_t[:], in_=b3_d[:])
            w2_t = []
            for k in range(KT):
                t = cpool.tile([128, HID], BF16, tag=f"w2{k}", name=f"w2{k}")
                nc.sync.dma_start(out=t[:], in_=w2_d[k])
                w2_t.append(t)

            ep_t = [cpool.tile([128, NE], BF16, tag=f"ep{k}", name=f"ep{k}")
                    for k in range(KT)]
            hp_t = [cpool.tile([128, N], F32, tag=f"hp{k}", name=f"hp{k}")
                    for k in range(KT)]

            # PE warm-up: dependency-free matmuls on memset data issue while
            # the weight DMAs are in flight so the HAM clock-gate reaches 8/8
            # before the first real matmul. Once per NEFF (outside the loop).
            warm_t = cpool.tile([128, 512], BF16, tag="warm")
            nc.vector.memset(warm_t[:], 0.5)
            for _ in range(warm):
                psw = y_ps.tile([128, 512], F32, tag="Y", name="psW")[:]
                nc.tensor.matmul(psw, warm_t[:, 0:128], warm_t[:],
                                 start=True, stop=True)

            def build_x(blk):
                """DVE: X[k][:, j*256:(j+1)*256] = relu(ep[k] + hp[k][:, n])
                for the two n's of the block. bf16 SBUF->SBUF dense -> 4x."""
                xt = []
                for k in range(KT):
                    t = xpool.tile([128, 512], BF16, tag=f"x{k}", name=f"x{k}")
                    for jj in range(2):
                        n = 2 * blk + jj
                        nc.vector.tensor_scalar(
                            out=t[:, ts(jj, NE)],
                            in0=ep_t[k][:],
                            scalar1=hp_t[k][:, n : n + 1],
                            scalar2=0.0,
                            op0=add,
                            op1=amax,
                        )
                    xt.append(t)
                return xt

            def body():
                # ---- preamble: epT (with b1 via aug row) and hpT ----
                for k in range(KT):
                    ps = y_ps.tile([128, NE], F32, tag="Y", name="psE")[:]
                    nc.tensor.matmul(ps, we_t[:, ts(k, 128)], et_t[:],
                                     start=True, stop=True)
                    nc.vector.tensor_copy(out=ep_t[k][:], in_=ps)
                for k in range(KT):
                    ps = y_ps.tile([128, N], F32, tag="Y", name="psH")[:]
                    for h in range(HT):
                        nc.tensor.matmul(
                            ps,
                            wh_t[h][:, ts(k, 128)],
                            ht_t[h][:],
                            start=(h == 0),
                            stop=(h == HT - 1),
                        )
                    nc.vector.tensor_copy(out=hp_t[k][:], in_=ps)

                xt = build_x(0)

                # pending[g] = psum strip tile for output group g
                pso4 = None
                ssum_prev = None     # (blk, ssum tile) awaiting reduce-MM
                drain_q = []         # (group, pso4 tile) awaiting ACT drain

                def reduce_prev():
                    nonlocal pso4, ssum_prev
                    if ssum_prev is None:
                        return
                    b_, ss_ = ssum_prev
                    bi = b_ % OG
                    if bi == 0:
                        pso4 = o_ps.tile([128, 512], F32, tag="po4",
                                         name="po4")
                    nc.tensor.matmul(
                        pso4[32 * bi : 32 * bi + 1, :],
                        vst_t[:],
                        ss_[:],
                        start=True,
                        stop=True,
                        tile_position=(0, 32 * bi),
                    )
                    if bi == OG - 1:
                        drain_q.append((b_ // OG, pso4))
                    ssum_prev = None

                def drain_out():
                    # one ACT op + 4 DMAs per group of OG blocks
                    while drain_q:
                        g, ps4 = drain_q.pop(0)
                        hi = 32 * (OG - 1) + 1
                        otg = opool.tile([hi, 512], F32, tag="og", name="og")
                        nc.scalar.activation(out=otg[:], in_=ps4[0:hi, :],
                                             func=ident, bias=b3_t[0:hi, :],
                                             scale=1.0)
                        for bi in range(OG):
                            nc.sync.dma_start(
                                out=out_d[OG * g + bi : OG * g + bi + 1, :],
                                in_=otg[32 * bi : 32 * bi + 1, :])

                for blk in range(NBLK):
                    yts = {}
                    acc = None
                    for gi, j in enumerate(JORD):
                        psy = y_ps.tile([128, 512], F32, tag="Y")
                        for k in range(KT):
                            nc.tensor.matmul(
                                psy[:],
                                w2_t[k][:, ts(j, 128)],
                                xt[k][:],
                                start=(k == 0),
                                stop=(k == KT - 1),
                            )
                        if gi == red_at:
                            reduce_prev()
                        yt = ypool.tile([128, 512], BF16, tag=f"y{j}",
                                        name=f"y{j}")
                        if j < 3:
                            nc.scalar.activation(
                                out=yt[:], in_=psy[:], func=relu,
                                bias=bias2_t[:, j : j + 1],
                                scale=scl_t[:, j : j + 1],
                            )
                        else:
                            nc.scalar.activation(
                                out=yt[:], in_=psy[:], func=relu,
                                bias=bias2_t[:, j : j + 1],
                                scale=1.0,
                            )
                        yts[j] = yt
                        if j == 3:
                            # signed w3 for the mixed tile; sign s(p) folded
                            # in so the +-1 reduce stationary cancels it
                            zt = spool.tile([128, 512], BF16, tag="z3",
                                            name="z3")
                            nc.vector.tensor_scalar(
                                out=zt[:], in0=yt[:],
                                scalar1=w3s3_t[:, 0:1],
                                scalar2=None, op0=mult,
                            )
                            yts["z3"] = zt
                            # X for the next block: DVE is free now; these
                            # land before the add chain in the DVE queue
                            if blk + 1 < NBLK:
                                xt_next = build_x(blk + 1)
                            drain_out()
                        if j == 0:
                            a1 = spool.tile([128, 512], BF16, tag="a1",
                                            name="a1")
                            nc.vector.tensor_add(out=a1[:], in0=yts["z3"][:],
                                                 in1=yts[0][:])
                        if j == 1:
                            a2 = spool.tile([128, 512], BF16, tag="a2",
                                            name="a2")
                            nc.vector.tensor_add(out=a2[:], in0=a1[:],
                                                 in1=yts[1][:])
                        if j == 2:
                            ss = spool.tile([128, 512], BF16, tag="ss",
                                            name="ss")
                            nc.vector.tensor_add(out=ss[:], in0=a2[:],
                                                 in1=yts[2][:])
                            ssum_prev = (blk, ss)
                    if blk + 1 < NBLK:
                        xt = xt_next
                # flush the last block's reduce + final output group
                reduce_prev()
                drain_out()

            if dyn_repeat is not None:
                hint = (mybir.EngineType.PE, mybir.EngineType.DVE,
                        mybir.EngineType.Activation)
                with tc.For_i(0, dyn_repeat, 1, hint_engines=hint):
                    body()
            else:
                for _rep in range(repeat):
                    body()
    return nc


def make_in_maps(h_all, e_feat, W1, b1, W2, b2, W3, b3):
    bf = ml_dtypes.bfloat16
    Wh = np.ascontiguousarray(W1[:H]).astype(bf).reshape(HT, 128, HID)
    We_aug = np.concatenate([W1[H:], b1[None, :]], axis=0).astype(bf)
    eT_aug = np.concatenate(
        [e_feat.T, np.ones((1, NE), np.float32)], axis=0
    ).astype(bf)

    # --- s3z permutation: lanes of tiles 0..2 sign-pure, tile 3 leftover ---
    w3v = np.asarray(W3, np.float32).reshape(-1)
    b2v = np.asarray(b2, np.float32).reshape(-1)
    pos = list(np.nonzero(w3v > 0)[0])
    neg = list(np.nonzero(w3v <= 0)[0])
    npos = len(pos)
    nneg = len(neg)
    # x = number of positive lanes; feasible for any npos since the interval
    # [128 - nneg//3, npos//3] has width 512/3 - 128 > 1
    x = min(npos // 3, 128)
    if 3 * (128 - x) > nneg:
        x = 128 - nneg // 3
    assert 0 <= x <= 128 and 3 * x <= npos and 3 * (128 - x) <= nneg
    pi = np.empty(HID, np.int64)
    for p in range(128):
        src = pos if p < x else neg
        for j in range(3):
            pi[j * 128 + p] = src.pop()
    rest = pos + neg
    pi[3 * 128 :] = rest
    sgn = np.where(np.arange(128) < x, 1.0, -1.0).astype(np.float32)

    W2p = np.asarray(W2, np.float32)[:, pi]
    w3p = w3v[pi]
    b2p = b2v[pi]
    scl = np.ones((128, JT), np.float32)
    bias2 = np.zeros((128, JT), np.float32)
    for j in range(JT):
        if j < 3:
            scl[:, j] = np.abs(w3p[j * 128 : (j + 1) * 128])
        bias2[:, j] = scl[:, j] * b2p[j * 128 : (j + 1) * 128]
    w3s3 = (w3p[3 * 128 :] * sgn).astype(np.float32).reshape(128, 1)
    vst = sgn.astype(bf).reshape(128, 1)

    W2k = np.ascontiguousarray(W2p).astype(bf).reshape(KT, 128, HID)
    b3c = np.ascontiguousarray(
        np.broadcast_to(np.asarray(b3, np.float32).reshape(1, 1), (128, 1))
    )
    shared = {
        "wh": Wh, "we": We_aug, "et": eT_aug, "w2": W2k,
        "scl": scl, "bias2": bias2, "w3s3": w3s3, "vst": vst, "b3": b3c,
    }
    in_maps = []
    for b in range(B):
        hT = np.ascontiguousarray(h_all[b].T).astype(bf).reshape(HT, 128, N)
        in_maps.append({"ht": hT, **shared})
    return in_maps


_nc_cache = {}

_CONFIG = {}


def _get_nc():
    if "nc" not in _nc_cache:
        nc = bacc.Bacc("TRN2", target_bir_lowering=False, debug=False,
                       num_devices=B)
        build(nc, **_CONFIG)
        nc.compile()
        _nc_cache["nc"] = nc
    return _nc_cache["nc"]


def kernel(h_all, e_feat, W1, b1, W2, b2, W3, b3):
    h_all = np.asarray(h_all, np.float32)
    e_feat = np.asarray(e_feat, np.float32)
    W1 = np.asarray(W1, np.float32)
    b1 = np.asarray(b1, np.float32)
    W2 = np.asarray(W2, np.float32)
    b2 = np.asarray(b2, np.float32)
    W3 = np.asarray(W3, np.float32)
    b3 = np.asarray(b3, np.float32)

    nc = _get_nc()
    in_maps = make_in_maps(h_all, e_feat, W1, b1, W2, b2, W3, b3)
    res = run_bass_kernel_spmd(nc, in_maps, core_ids=list(range(B)))
    out = np.stack([res.results[i]["out"].reshape(N, NE, OUT) for i in range(B)])
    return out.astype(np.float32)


# revision 20
# speedup vs baseline: 1.1675x; 1.1675x over previous
"""Trainium2 Bass kernel for AllAtomEnergyBranch (3-layer MLP over broadcast concat).

Math (per batch b, position n, edge e):
    out[b,n,e,0] = W3^T relu(W2^T relu(Wh^T h[b,n] + We^T e_feat[e] + b1) + b2) + b3

Sharding: data-parallel over B (8 batches -> 8 NeuronCores), weights replicated.
Each core computes its [64, 256] output slice independently; no collectives.

Per-core dataflow ("s3z" scheme, software-pipelined):
  - preamble: epT [512k, 256e] = We_aug.T @ eT_aug (b1 folded via ones row),
    hpT [512k, 64n] = Wh.T @ hT.  X(0) built before the block loop.
  - hidden-2 units are PERMUTED on host so that within each PE lane p the
    three units assigned to j-tiles 0..2 share one sign s(p); |w3| (and
    |w3|*b2) are folded into the ACT relu scale/bias for those tiles, and
    the sign rides the [128,1] +-1 stationary of the per-block reduce
    matmul.  Tile 3 takes the leftover mixed-sign units: plain relu on ACT,
    then one DVE multiply by (w3*s) so the sign cancels in the reduce.
    This removes 3 of the 4 per-block DVE w3-multiplies of the baseline at
    zero PE cost (still exactly one reduce matmul per block) and works for
    any sign pattern / any b2.
  - per block of 2 n's (32 blocks), emission software-pipelined:
      j-order (3,0,1,2); X(b+1) DVE builds hoisted between; reduce-MM(b-1)
      interleaved after block b's second matmul group; output drained via
      ACT once per 4 blocks from a strip-batched PSUM tile (reduce matmuls
      target partitions 0/32/64/96 by tile_position).
  - xpre: each iteration's tail emits the NEXT iteration's preamble and
    X(0) (values identical across iterations), so the next body's matmul
    stream starts immediately.
  - benchmark For_i loop unrolls several bodies per hardware iteration:
    For_i inserts an ALL-ENGINE BARRIER in its per-iteration reset block,
    so unrolling amortizes the barrier + tail-drain (measured -8us/iter
    at unroll=4 on silicon).
  - engine budget per block (spec): PE 17 matmuls ~3.7us, ACT 4 relus +
    out/4 ~2.4us, DVE 8 X-builds + 1 mult + 3 adds ~2.2us.  PE-bound.

Measured on silicon (interleaved min-wall delta of For_i NEFFs, same
process, same chip state): previous-session baseline 158.5-186.3us,
this kernel 126.0us at unroll=16+pair (vs 132.6 at unroll=8, 150.7 at
unroll=4, 155.8 at unroll=1); isolated PE stream (matmuls only)
130.5-148.3us in the same states -- the sustained-throttle PE clock
(~2.0 GHz vs 2.4 nominal) is the wall; the 512 per-iteration layer-2
matmuls are irreducible at bf16 (fp8 DoubleRow fails the 2e-2 gate:
emulated rel err 4.1%).
rel err vs f32 reference: 0.0048 (harness gate 2e-2).

Rejected on measurement: GPSIMD partition_all_reduce for the partition
sum (+43%: Q7 daisy-chain too slow, DVE port contention), extra narrow
reduce matmuls, act2-style 1024-wide relu pairing (tie), red_at=2 (tie).
"""

import numpy as np
import ml_dtypes

import concourse.bass as bass
import concourse.bass_isa as bass_isa
import concourse.mybir as mybir
from concourse import bacc
from concourse.bass import ts
from concourse.tile import TileContext
from concourse.bass_utils import run_bass_kernel_spmd

BF16 = mybir.dt.bfloat16
F32 = mybir.dt.float32

B, N, H = 8, 64, 256
NE, E = 256, 64
HID, OUT = 512, 1
KT = HID // 128   # 4 k-tiles of layer-1 output / layer-2 contraction
JT = HID // 128   # 4 j-tiles of layer-2 output / layer-3 contraction
HT = H // 128     # 2 h-tiles of layer-1 contraction
NBLK = N // 2     # blocks of 2 n-values -> 512 moving columns per matmul
OG = 4            # output blocks batched per PSUM strip-tile / ACT drain


def build(nc, repeat=1, dyn_repeat=None, y_bufs=6, x_bufs=2, yt_bufs=2,
          red_at=1, warm=8, stage="full", xpre=1, pair=0, dma_out=0,
          red_eng="pe", unroll=1):
    """Build the per-core graph (s3z scheme). All 8 cores run the same
    program.

    repeat / dyn_repeat: repeat the whole computation inside the NEFF
    (python-unrolled / For_i hardware loop) -- benchmarking only.
    red_at: which j-group of block b+1 the reduce-MM of block b is emitted
    after (0..3); ssum(b) lands ~1 group into block b+1.
    pair: process 4 n's per macro-block (1024-col moving operands / 2-bank
    PSUM tiles) instead of 2 -- halves instruction counts on every engine.
    """
    nb = 4 if pair else 2          # n's per macro-block
    colw = 256 * nb                # moving columns per matmul
    NB = N // nb                   # macro-blocks per iteration
    ht_d = nc.declare_dram_parameter("ht", [HT, 128, N], BF16, isOutput=False)
    wh_d = nc.declare_dram_parameter("wh", [HT, 128, HID], BF16, isOutput=False)
    we_d = nc.declare_dram_parameter("we", [E + 1, HID], BF16, isOutput=False)
    et_d = nc.declare_dram_parameter("et", [E + 1, NE], BF16, isOutput=False)
    w2_d = nc.declare_dram_parameter("w2", [KT, 128, HID], BF16, isOutput=False)
    scl_d = nc.declare_dram_parameter("scl", [128, JT], F32, isOutput=False)
    bias2_d = nc.declare_dram_parameter("bias2", [128, JT], F32, isOutput=False)
    w3s3_d = nc.declare_dram_parameter("w3s3", [128, 1], F32, isOutput=False)
    vst_d = nc.declare_dram_parameter("vst", [128, 1], BF16, isOutput=False)
    b3_d = nc.declare_dram_parameter("b3", [128, 1], F32, isOutput=False)
    cfold_d = nc.declare_dram_parameter("cfold", [128, 1], F32, isOutput=False)
    vstf_d = nc.declare_dram_parameter("vstf", [128, 1], F32, isOutput=False)
    cg_d = nc.declare_dram_parameter("cg", [128, 1], F32, isOutput=False)
    out_d = nc.declare_dram_parameter("out", [NBLK, 512], F32, isOutput=True)

    relu = mybir.ActivationFunctionType.Relu
    ident = mybir.ActivationFunctionType.Identity
    add = mybir.AluOpType.add
    mult = mybir.AluOpType.mult
    amax = mybir.AluOpType.max

    JORD = (3, 0, 1, 2)   # tile 3 first so z3 and the add chain start early

    with TileContext(nc) as tc:
        with (
            tc.tile_pool(name="const", bufs=1) as cpool,
            tc.tile_pool(name="xp", bufs=x_bufs) as xpool,
            tc.tile_pool(name="yp", bufs=yt_bufs) as ypool,
            tc.tile_pool(name="sp", bufs=3) as spool,
            tc.tile_pool(name="op", bufs=2) as opool,
            tc.tile_pool(name="ar", bufs=2) as arpool,
            tc.tile_pool(name="psY", bufs=(3 if pair else y_bufs),
                         space="PSUM") as y_ps,
            tc.tile_pool(name="psO", bufs=2, space="PSUM") as o_ps,
        ):
            # ---- load weights / inputs into SBUF ----
            # Preamble operands (we/et/ht/wh) first so the PE can start
            # while W2 is still in flight.
            we_t = cpool.tile([E + 1, HID], BF16, tag="we")
            nc.sync.dma_start(out=we_t[:], in_=we_d[:])
            et_t = cpool.tile([E + 1, NE], BF16, tag="et")
            nc.sync.dma_start(out=et_t[:], in_=et_d[:])
            ht_t = []
            for h in range(HT):
                t = cpool.tile([128, N], BF16, tag=f"ht{h}", name=f"ht{h}")
                nc.sync.dma_start(out=t[:], in_=ht_d[h])
                ht_t.append(t)
            wh_t = []
            for h in range(HT):
                t = cpool.tile([128, HID], BF16, tag=f"wh{h}", name=f"wh{h}")
                nc.sync.dma_start(out=t[:], in_=wh_d[h])
                wh_t.append(t)
            scl_t = cpool.tile([128, JT], F32, tag="scl")
            nc.sync.dma_start(out=scl_t[:], in_=scl_d[:])
            bias2_t = cpool.tile([128, JT], F32, tag="bias2")
            nc.sync.dma_start(out=bias2_t[:], in_=bias2_d[:])
            w3s3_t = cpool.tile([128, 1], F32, tag="w3s3")
            nc.sync.dma_start(out=w3s3_t[:], in_=w3s3_d[:])
            vst_t = cpool.tile([128, 1], BF16, tag="vst")
            nc.sync.dma_start(out=vst_t[:], in_=vst_d[:])
            b3_t = cpool.tile([128, 1], F32, tag="b3")
            nc.sync.dma_start(out=b3_t[:], in_=b3_d[:])
            cfold_t = cpool.tile([128, 1], F32, tag="cfold")
            nc.sync.dma_start(out=cfold_t[:], in_=cfold_d[:])
            vstf_t = cpool.tile([128, 1], F32, tag="vstf")
            nc.sync.dma_start(out=vstf_t[:], in_=vstf_d[:])
            cg_t = cpool.tile([128, 1], F32, tag="cg")
            nc.sync.dma_start(out=cg_t[:], in_=cg_d[:])
            w2_t = []
            for k in range(KT):
                t = cpool.tile([128, HID], BF16, tag=f"w2{k}", name=f"w2{k}")
                nc.sync.dma_start(out=t[:], in_=w2_d[k])
                w2_t.append(t)

            ep_t = [cpool.tile([128, NE], BF16, tag=f"ep{k}", name=f"ep{k}")
                    for k in range(KT)]
            hp_t = [cpool.tile([128, N], F32, tag=f"hp{k}", name=f"hp{k}")
                    for k in range(KT)]

            # PE warm-up: dependency-free matmuls on memset data issue while
            # the weight DMAs are in flight so the HAM clock-gate reaches 8/8
            # before the first real matmul. Once per NEFF (outside the loop).
            warm_t = cpool.tile([128, 512], BF16, tag="warm")
            nc.vector.memset(warm_t[:], 0.5)
            for _ in range(warm):
                psw = y_ps.tile([128, 512], F32, tag="Y", name="psW")[:]
                nc.tensor.matmul(psw, warm_t[:, 0:128], warm_t[:],
                                 start=True, stop=True)

            def build_x(blk):
                """DVE: X[k][:, jj*256:(jj+1)*256] = relu(ep[k] + hp[k][:, n])
                for the nb n's of the block. bf16 SBUF->SBUF dense -> 4x."""
                xt = []
                for k in range(KT):
                    t = xpool.tile([128, colw], BF16, tag=f"x{k}", name=f"x{k}")
                    for jj in range(nb):
                        n = nb * blk + jj
                        nc.vector.tensor_scalar(
                            out=t[:, ts(jj, NE)],
                            in0=ep_t[k][:],
                            scalar1=hp_t[k][:, n : n + 1],
                            scalar2=0.0,
                            op0=add,
                            op1=amax,
                        )
                    xt.append(t)
                return xt

            def body_stage():
                # engine-isolated streams for rate measurement (bench only)
                xt = build_x(0) if stage != "dve" else None
                if stage == "pe":
                    for blk in range(NBLK):
                        for j in JORD:
                            psy = y_ps.tile([128, 512], F32, tag="Y")
                            for k in range(KT):
                                nc.tensor.matmul(
                                    psy[:], w2_t[k][:, ts(j, 128)], xt[k][:],
                                    start=(k == 0), stop=(k == KT - 1),
                                )
                        pso = o_ps.tile([128, 512], F32, tag="po4")
                        nc.tensor.matmul(pso[0:1, :], vst_t[:], warm_t[:],
                                         start=True, stop=True)
                elif stage == "dve":
                    zs = spool.tile([128, 512], BF16, tag="zs", name="zs")
                    nc.vector.memset(zs[:], 0.25)
                    for blk in range(NBLK):
                        xt = build_x(blk)
                        zt = spool.tile([128, 512], BF16, tag="z3")
                        nc.vector.tensor_scalar(
                            out=zt[:], in0=zs[:], scalar1=w3s3_t[:, 0:1],
                            scalar2=None, op0=mult)
                        a1 = spool.tile([128, 512], BF16, tag="a1")
                        nc.vector.tensor_add(out=a1[:], in0=zt[:], in1=zs[:])
                        a2 = spool.tile([128, 512], BF16, tag="a2")
                        nc.vector.tensor_add(out=a2[:], in0=a1[:], in1=zs[:])
                        ss = spool.tile([128, 512], BF16, tag="ss")
                        nc.vector.tensor_add(out=ss[:], in0=a2[:], in1=zs[:])
                elif stage == "act":
                    psy = y_ps.tile([128, 512], F32, tag="Y")
                    for k in range(KT):
                        nc.tensor.matmul(psy[:], w2_t[k][:, ts(0, 128)],
                                         xt[k][:], start=(k == 0),
                                         stop=(k == KT - 1))
                    pso = o_ps.tile([128, 512], F32, tag="po4")
                    nc.tensor.matmul(pso[0:1, :], vst_t[:], warm_t[:],
                                     start=True, stop=True)
                    for blk in range(NBLK):
                        for j in JORD:
                            yt = ypool.tile([128, 512], BF16, tag=f"y{j}")
                            nc.scalar.activation(
                                out=yt[:], in_=psy[:], func=relu,
                                bias=bias2_t[:, j : j + 1],
                                scale=scl_t[:, j : j + 1] if j < 3 else 1.0)
                        if blk % OG == OG - 1:
                            otg = opool.tile([97, 512], F32, tag="og")
                            nc.scalar.activation(
                                out=otg[:], in_=pso[0:97, :], func=ident,
                                bias=b3_t[0:97, :], scale=1.0)

            def preamble():
                # epT (with b1 via aug row) and hpT
                for k in range(KT):
                    ps = y_ps.tile([128, NE], F32, tag="Y", name="psE")[:]
                    nc.tensor.matmul(ps, we_t[:, ts(k, 128)], et_t[:],
                                     start=True, stop=True)
                    nc.vector.tensor_copy(out=ep_t[k][:], in_=ps)
                for k in range(KT):
                    ps = y_ps.tile([128, N], F32, tag="Y", name="psH")[:]
                    for h in range(HT):
                        nc.tensor.matmul(
                            ps,
                            wh_t[h][:, ts(k, 128)],
                            ht_t[h][:],
                            start=(h == 0),
                            stop=(h == HT - 1),
                        )
                    nc.vector.tensor_copy(out=hp_t[k][:], in_=ps)

            # xpre: iteration i+1's preamble and X(0) are emitted at the END
            # of iteration i (values are identical across iterations), so the
            # next iteration's first matmul group never waits on the
            # preamble->copy->X chain.  Iteration 0 uses this hoisted copy.
            xt_hold = {}
            if xpre:
                preamble()
                xt_hold["xt"] = build_x(0)

            def body():
                if stage != "full":
                    return body_stage()
                if xpre:
                    xt = xt_hold["xt"]
                else:
                    preamble()
                    xt = build_x(0)

                # reduce/drain operate on 512-col strips ("blk512" units)
                pso4 = None
                pending = []         # (blk512, ssum AP) awaiting reduce-MM
                drain_q = []         # (group, pso4 tile) awaiting ACT drain

                def reduce_prev():
                    nonlocal pso4
                    while pending:
                        b_, ss_ = pending.pop(0)
                        bi = b_ % OG
                        if bi == 0:
                            pso4 = o_ps.tile([128, 512], F32, tag="po4",
                                             name="po4")
                        nc.tensor.matmul(
                            pso4[32 * bi : 32 * bi + 1, :],
                            vst_t[:],
                            ss_,
                            start=True,
                            stop=True,
                            tile_position=(0, 32 * bi),
                        )
                        if bi == OG - 1:
                            drain_q.append((b_ // OG, pso4))

                def drain_out():
                    while drain_q:
                        g, ps4 = drain_q.pop(0)
                        if dma_out:
                            # b3 was folded into z3 (c = b3 / sum(s)); one
                            # partition-strided DMA straight from PSUM
                            nc.sync.dma_start(
                                out=out_d[OG * g : OG * (g + 1), :],
                                in_=ps4[0 : 32 * (OG - 1) + 1 : 32, :])
                        else:
                            hi = 32 * (OG - 1) + 1
                            otg = opool.tile([hi, 512], F32, tag="og",
                                             name="og")
                            nc.scalar.activation(out=otg[:], in_=ps4[0:hi, :],
                                                 func=ident,
                                                 bias=b3_t[0:hi, :],
                                                 scale=1.0)
                            for bi in range(OG):
                                nc.sync.dma_start(
                                    out=out_d[OG * g + bi : OG * g + bi + 1, :],
                                    in_=otg[32 * bi : 32 * bi + 1, :])

                for blk in range(NB):
                    yts = {}
                    for gi, j in enumerate(JORD):
                        psy = y_ps.tile([128, colw], F32, tag="Y")
                        # 512-col accumulation groups (one PSUM bank each);
                        # walrus rejects 2-bank matmul outputs
                        for h2 in range(nb // 2):
                            for k in range(KT):
                                nc.tensor.matmul(
                                    psy[:, ts(h2, 512)],
                                    w2_t[k][:, ts(j, 128)],
                                    xt[k][:, ts(h2, 512)],
                                    start=(k == 0),
                                    stop=(k == KT - 1),
                                )
                        if gi == red_at:
                            reduce_prev()
                        yt = ypool.tile([128, colw], BF16, tag=f"y{j}",
                                        name=f"y{j}")
                        if j < 3:
                            nc.scalar.activation(
                                out=yt[:], in_=psy[:], func=relu,
                                bias=bias2_t[:, j : j + 1],
                                scale=scl_t[:, j : j + 1],
                            )
                        else:
                            nc.scalar.activation(
                                out=yt[:], in_=psy[:], func=relu,
                                bias=bias2_t[:, j : j + 1],
                                scale=1.0,
                            )
                        yts[j] = yt
                        if j == 3:
                            # signed w3 for the mixed tile; sign s(p) folded
                            # in so the +-1 reduce stationary cancels it
                            zt = spool.tile([128, colw], BF16, tag="z3",
                                            name="z3")
                            if dma_out:
                                # z3 = y3 * (w3*s) + c, with c = b3/sum(s) so
                                # the +-1 reduce recovers "+ b3" exactly
                                nc.vector.tensor_scalar(
                                    out=zt[:], in0=yt[:],
                                    scalar1=w3s3_t[:, 0:1],
                                    scalar2=cfold_t[:, 0:1],
                                    op0=mult, op1=add,
                                )
                            else:
                                nc.vector.tensor_scalar(
                                    out=zt[:], in0=yt[:],
                                    scalar1=w3s3_t[:, 0:1],
                                    scalar2=None, op0=mult,
                                )
                            yts["z3"] = zt
                            # X for the next block: DVE is free now; these
                            # land before the add chain in the DVE queue
                            if blk + 1 < NB:
                                xt_next = build_x(blk + 1)
                            drain_out()
                        if j == 0:
                            a1 = spool.tile([128, colw], BF16, tag="a1",
                                            name="a1")
                            nc.vector.tensor_add(out=a1[:], in0=yts["z3"][:],
                                                 in1=yts[0][:])
                        if j == 1:
                            a2 = spool.tile([128, colw], BF16, tag="a2",
                                            name="a2")
                            nc.vector.tensor_add(out=a2[:], in0=a1[:],
                                                 in1=yts[1][:])
                        if j == 2:
                            ss = spool.tile([128, colw], BF16, tag="ss",
                                            name="ss")
                            nc.vector.tensor_add(out=ss[:], in0=a2[:],
                                                 in1=yts[2][:])
                            if red_eng == "gps":
                                # sign + b3/128 fold (DVE), then the idle
                                # GPSIMD does the partition sum off the PE
                                ssg = spool.tile([128, colw], BF16, tag="ssg",
                                                 name="ssg")
                                nc.vector.tensor_scalar(
                                    out=ssg[:], in0=ss[:],
                                    scalar1=vstf_t[:, 0:1],
                                    scalar2=cg_t[:, 0:1],
                                    op0=mult, op1=add,
                                )
                                ar = arpool.tile([128, colw], F32, tag="ar",
                                                 name="ar")
                                nc.gpsimd.partition_all_reduce(
                                    ar[:], ssg[:], 128,
                                    bass_isa.ReduceOp.add)
                                for h2 in range(nb // 2):
                                    b512 = (nb // 2) * blk + h2
                                    nc.sync.dma_start(
                                        out=out_d[b512 : b512 + 1, :],
                                        in_=ar[0:1, ts(h2, 512)])
                            else:
                                for h2 in range(nb // 2):
                                    pending.append(
                                        ((nb // 2) * blk + h2,
                                         ss[:, ts(h2, 512)]))
                    if blk + 1 < NB:
                        xt = xt_next
                # flush the last block's reduce + final output group
                reduce_prev()
                drain_out()
                if xpre:
                    # next iteration's X(0) (reads current ep/hp), then its
                    # preamble (overwrites ep/hp after that read)
                    xt_hold["xt"] = build_x(0)
                    preamble()

            if dyn_repeat is not None:
                # For_i puts an all-engine barrier in each iteration's reset
                # block (no cross-iteration overlap); unrolling several
                # bodies per hardware iteration amortizes barrier + drain.
                assert dyn_repeat % unroll == 0
                hint = (mybir.EngineType.PE, mybir.EngineType.DVE,
                        mybir.EngineType.Activation)
                with tc.For_i(0, dyn_repeat // unroll, 1, hint_engines=hint):
                    for _u in range(unroll):
                        body()
            else:
                for _rep in range(repeat):
                    body()
    return nc


def make_in_maps(h_all, e_feat, W1, b1, W2, b2, W3, b3):
    bf = ml_dtypes.bfloat16
    Wh = np.ascontiguousarray(W1[:H]).astype(bf).reshape(HT, 128, HID)
    We_aug = np.concatenate([W1[H:], b1[None, :]], axis=0).astype(bf)
    eT_aug = np.concatenate(
        [e_feat.T, np.ones((1, NE), np.float32)], axis=0
    ).astype(bf)

    # --- s3z permutation: lanes of tiles 0..2 sign-pure, tile 3 leftover ---
    w3v = np.asarray(W3, np.float32).reshape(-1)
    b2v = np.asarray(b2, np.float32).reshape(-1)
    pos = list(np.nonzero(w3v > 0)[0])
    neg = list(np.nonzero(w3v <= 0)[0])
    npos = len(pos)
    nneg = len(neg)
    # x = number of positive lanes; feasible for any npos since the interval
    # [128 - nneg//3, npos//3] has width 512/3 - 128 > 1
    x = min(npos // 3, 128)
    if 3 * (128 - x) > nneg:
        x = 128 - nneg // 3
    b3v = float(np.asarray(b3, np.float32).reshape(-1)[0])
    if x == 64 and b3v != 0.0:
        # need sum(s) != 0 to fold b3 through the +-1 reduce
        if 3 * (x + 1) <= npos:
            x += 1
        else:
            x -= 1
    assert 0 <= x <= 128 and 3 * x <= npos and 3 * (128 - x) <= nneg
    pi = np.empty(HID, np.int64)
    for p in range(128):
        src = pos if p < x else neg
        for j in range(3):
            pi[j * 128 + p] = src.pop()
    rest = pos + neg
    pi[3 * 128 :] = rest
    sgn = np.where(np.arange(128) < x, 1.0, -1.0).astype(np.float32)

    W2p = np.asarray(W2, np.float32)[:, pi]
    w3p = w3v[pi]
    b2p = b2v[pi]
    scl = np.ones((128, JT), np.float32)
    bias2 = np.zeros((128, JT), np.float32)
    for j in range(JT):
        if j < 3:
            scl[:, j] = np.abs(w3p[j * 128 : (j + 1) * 128])
        bias2[:, j] = scl[:, j] * b2p[j * 128 : (j + 1) * 128]
    w3s3 = (w3p[3 * 128 :] * sgn).astype(np.float32).reshape(128, 1)
    vst = sgn.astype(bf).reshape(128, 1)
    ssgn = 2 * x - 128
    cval = 0.0 if b3v == 0.0 else b3v / ssgn
    cfold = np.full((128, 1), cval, np.float32)
    vstf = sgn.astype(np.float32).reshape(128, 1)
    cg = np.full((128, 1), b3v / 128.0, np.float32)

    W2k = np.ascontiguousarray(W2p).astype(bf).reshape(KT, 128, HID)
    b3c = np.ascontiguousarray(
        np.broadcast_to(np.asarray(b3, np.float32).reshape(1, 1), (128, 1))
    )
    shared = {
        "wh": Wh, "we": We_aug, "et": eT_aug, "w2": W2k,
        "scl": scl, "bias2": bias2, "w3s3": w3s3, "vst": vst, "b3": b3c,
        "cfold": cfold, "vstf": vstf, "cg": cg,
    }
    in_maps = []
    for b in range(B):
        hT = np.ascontiguousarray(h_all[b].T).astype(bf).reshape(HT, 128, N)
        in_maps.append({"ht": hT, **shared})
    return in_maps


_nc_cache = {}

_CONFIG = {"unroll": 16, "pair": 1}


def _get_nc():
    if "nc" not in _nc_cache:
        nc = bacc.Bacc("TRN2", target_bir_lowering=False, debug=False,
                       num_devices=B)
        build(nc, **_CONFIG)
        nc.compile()
        _nc_cache["nc"] = nc
    return _nc_cache["nc"]


def kernel(h_all, e_feat, W1, b1, W2, b2, W3, b3):
    h_all = np.asarray(h_all, np.float32)
    e_feat = np.asarray(e_feat, np.float32)
    W1 = np.asarray(W1, np.float32)
    b1 = np.asarray(b1, np.float32)
    W2 = np.asarray(W2, np.float32)
    b2 = np.asarray(b2, np.float32)
    W3 = np.asarray(W3, np.float32)
    b3 = np.asarray(b3, np.float32)

    nc = _get_nc()
    in_maps = make_in_maps(h_all, e_feat, W1, b1, W2, b2, W3, b3)
    res = run_bass_kernel_spmd(nc, in_maps, core_ids=list(range(B)))
    out = np.stack([res.results[i]["out"].reshape(N, NE, OUT) for i in range(B)])
    return out.astype(np.float32)
